# revision 5
# baseline (speedup 1.0000x reference)
# Trainium2 Bass kernel for nn_CounterfactualGenerator (gnn_message_passing).
#
# Strategy:
#   * Pure data parallel over the batch dim B across 8 NeuronCores.
#   * The sequential 3x2048-edge propagation scan is a LINEAR operator on each
#     row's V-vector: every edge update d[:,e] += d[:,c]*(s*0.1) is an
#     elementary matrix A_i = I + coef*E_{c,e}; the full scan is d0 @ (A_1...A_N).
#     We form M = prod A_i on the host in float64 (O(E*V) work, ~ms) and apply
#     it on-device as one [B,V]x[V,V] matmul.  To keep fp32 accuracy with bf16
#     matmuls we split M = I + Mc and compute cf = pre + pre@Mc (the identity
#     part stays exact fp32; only the correction sees bf16 rounding).
#   * Everything on device lives in "transposed" layout [feature, batch] so
#     that all matmuls contract over SBUF partitions with batch streaming along
#     the free dim (N=512 chunks, one PSUM bank per matmul).
#   * Host does: input transpose/shard, M precompute, final transpose back,
#     sigmoid (plaus) and sqrt (impact) on [B]-sized vectors.
import sys

sys.path.insert(0, "/opt/trn_rl_repo")

import numpy as np
import ml_dtypes

import concourse.bass as bass
import concourse.mybir as mybir
import concourse.tile as tile
from concourse import bacc
from concourse.bass_utils import run_bass_kernel_spmd

NCORES = 8
V = 256
K = 4
NB = 512  # matmul free-dim chunk (one PSUM bank)
PROP_SCALE = 0.1
INTERVENTION_STRENGTH = 1.0

F32 = mybir.dt.float32
BF16 = mybir.dt.bfloat16
bf16 = ml_dtypes.bfloat16

_KERNEL_CACHE = {}


def _iv_runs(ivars):
    """Group intervention variable indices into maximal consecutive runs that
    stay within one 128-partition chunk. Returns [(chunk, row_start, iv_off, n)].
    Write order preserved (matters only for duplicate indices)."""
    runs = []
    i = 0
    ivars = list(ivars)
    while i < len(ivars):
        j = i + 1
        while (j < len(ivars) and ivars[j] == ivars[j - 1] + 1
               and ivars[j] // 128 == ivars[i] // 128):
            j += 1
        runs.append((ivars[i] // 128, ivars[i] % 128, i, j - i))
        i = j
    return runs


def _build_nc(Bc, has_b1, has_b2, has_pb1, has_pb2, iv_runs):
    """Build the per-core Bass module. Bc = batch elements per core."""
    nchunk = Bc // NB
    nc = bacc.Bacc("TRN2", target_bir_lowering=False, debug=False, num_devices=NCORES)

    # ---- DRAM I/O ----
    xT = nc.dram_tensor("xT", [V, Bc], F32, kind="ExternalInput")
    xTb = nc.dram_tensor("xTb", [V, Bc], BF16, kind="ExternalInput")
    ivT = nc.dram_tensor("ivT", [K, Bc], F32, kind="ExternalInput")
    ivTb = nc.dram_tensor("ivTb", [K, Bc], BF16, kind="ExternalInput")
    magTb = nc.dram_tensor("magTb", [K, Bc], BF16, kind="ExternalInput")
    w1t = nc.dram_tensor("W1T", [V, V], BF16, kind="ExternalInput")  # [vin, o]
    w2t = nc.dram_tensor("W2T", [V, V], BF16, kind="ExternalInput")  # [o, vout]
    mct = nc.dram_tensor("MC", [V, V], BF16, kind="ExternalInput")  # [vin, vout]
    selt = nc.dram_tensor("SEL", [K, V], BF16, kind="ExternalInput")
    pw1t = nc.dram_tensor("PW1T", [3 * V, 128], BF16, kind="ExternalInput")
    pw2t = nc.dram_tensor("PW2T", [128, 64], BF16, kind="ExternalInput")
    pw3t = nc.dram_tensor("PW3T", [64, 1], BF16, kind="ExternalInput")
    if has_b1:
        b1t = nc.dram_tensor("B1", [V, 1], F32, kind="ExternalInput")
    if has_b2:
        b2t = nc.dram_tensor("B2", [K, V], BF16, kind="ExternalInput")
    if has_pb1:
        pb1t = nc.dram_tensor("PB1", [128, 1], F32, kind="ExternalInput")
    if has_pb2:
        pb2t = nc.dram_tensor("PB2", [64, 1], F32, kind="ExternalInput")

    outC = nc.dram_tensor("outC", [V, Bc], F32, kind="ExternalOutput")  # final_cf^T
    outE = nc.dram_tensor("outE", [V, Bc], F32, kind="ExternalOutput")  # effects^T
    outP = nc.dram_tensor("outP", [1, Bc], F32, kind="ExternalOutput")  # plaus logits
    outI = nc.dram_tensor("outI", [1, Bc], F32, kind="ExternalOutput")  # impact^2

    AF = mybir.ActivationFunctionType

    with tile.TileContext(nc) as tc:
        with (
            tc.tile_pool(name="consts", bufs=1) as consts,
            tc.tile_pool(name="work", bufs=2) as work,
            tc.tile_pool(name="psum", bufs=1, space="PSUM") as psum,
        ):
            # ---- load replicated weights once ----
            w1 = [consts.tile([128, V], BF16, name=f"w1_{c}") for c in range(2)]
            w2 = [consts.tile([128, V], BF16, name=f"w2_{c}") for c in range(2)]
            mc = [consts.tile([128, V], BF16, name=f"mc_{c}") for c in range(2)]
            for c in range(2):
                nc.sync.dma_start(out=w1[c], in_=w1t[c * 128 : (c + 1) * 128, :])
                nc.sync.dma_start(out=w2[c], in_=w2t[c * 128 : (c + 1) * 128, :])
                nc.sync.dma_start(out=mc[c], in_=mct[c * 128 : (c + 1) * 128, :])
            sel = consts.tile([K, V], BF16, name="sel")
            nc.sync.dma_start(out=sel, in_=selt[:, :])
            pw1 = [consts.tile([128, 128], BF16, name=f"pw1_{i}") for i in range(6)]
            for i in range(6):
                nc.sync.dma_start(out=pw1[i], in_=pw1t[i * 128 : (i + 1) * 128, :])
            pw2 = consts.tile([128, 64], BF16, name="pw2")
            nc.sync.dma_start(out=pw2, in_=pw2t[:, :])
            pw3 = consts.tile([64, 1], BF16, name="pw3")
            nc.sync.dma_start(out=pw3, in_=pw3t[:, :])
            ones = consts.tile([128, 1], BF16, name="ones")
            nc.vector.memset(ones, 1.0)
            b1 = None
            if has_b1:
                b1 = [consts.tile([128, 1], F32, name=f"b1_{c}") for c in range(2)]
                for c in range(2):
                    nc.sync.dma_start(out=b1[c], in_=b1t[c * 128 : (c + 1) * 128, :])
            b2 = None
            if has_b2:
                b2 = consts.tile([K, V], BF16, name="b2")
                nc.sync.dma_start(out=b2, in_=b2t[:, :])
            pb1 = None
            if has_pb1:
                pb1 = consts.tile([128, 1], F32, name="pb1")
                nc.sync.dma_start(out=pb1, in_=pb1t[:, :])
            pb2 = None
            if has_pb2:
                pb2 = consts.tile([64, 1], F32, name="pb2")
                nc.sync.dma_start(out=pb2, in_=pb2t[:, :])

            # ---- per-chunk pipeline ----
            for j in range(nchunk):
                cols = slice(j * NB, (j + 1) * NB)

                xf, xb = [], []
                for c in range(2):
                    t = work.tile([128, NB], F32, name=f"xf{c}", tag=f"xf{c}")
                    nc.sync.dma_start(out=t, in_=xT[c * 128 : (c + 1) * 128, cols])
                    xf.append(t)
                    t = work.tile([128, NB], BF16, name=f"xb{c}", tag=f"xb{c}")
                    nc.sync.dma_start(out=t, in_=xTb[c * 128 : (c + 1) * 128, cols])
                    xb.append(t)
                ivf = work.tile([K, NB], F32, name="ivf", tag="ivf")
                nc.sync.dma_start(out=ivf, in_=ivT[:, cols])
                ivb = work.tile([K, NB], BF16, name="ivb", tag="ivb")
                nc.sync.dma_start(out=ivb, in_=ivTb[:, cols])
                magb = work.tile([K, NB], BF16, name="magb", tag="magb")
                nc.sync.dma_start(out=magb, in_=magTb[:, cols])

                # mag broadcast [K,NB] -> [256,NB] via selector matmul
                mf = [psum.tile([128, NB], F32, name=f"mf{c}", tag="ps", bufs=8)
                      for c in range(2)]
                for c in range(2):
                    nc.tensor.matmul(mf[c], sel[:, c * 128 : (c + 1) * 128], magb,
                                     start=True, stop=True)

                # h^T = relu(W1cat @ x + b1)  [256, NB]
                hp = [psum.tile([128, NB], F32, name=f"hp{c}", tag="ps", bufs=8)
                      for c in range(2)]
                for c in range(2):
                    nc.tensor.matmul(hp[c], w1[0][:, c * 128 : (c + 1) * 128], xb[0],
                                     start=True, stop=False)
                    nc.tensor.matmul(hp[c], w1[1][:, c * 128 : (c + 1) * 128], xb[1],
                                     start=False, stop=True)
                hr = []
                for c in range(2):
                    t = work.tile([128, NB], F32, name=f"hr{c}", tag=f"hr{c}")
                    nc.scalar.activation(t, hp[c], AF.Relu,
                                         bias=b1[c] if has_b1 else 0.0)
                    hr.append(t)
                # hs = h * mag_broadcast  (bf16)
                hs = []
                for c in range(2):
                    t = work.tile([128, NB], BF16, name=f"hs{c}", tag=f"hs{c}")
                    nc.vector.tensor_mul(t, hr[c], mf[c])
                    hs.append(t)

                # eff^T = W2cat @ hs (+ b2-part)  [256, NB]
                ep = [psum.tile([128, NB], F32, name=f"ep{c}", tag="ps", bufs=8)
                      for c in range(2)]
                for c in range(2):
                    nc.tensor.matmul(ep[c], w2[0][:, c * 128 : (c + 1) * 128], hs[0],
                                     start=True, stop=False)
                    nc.tensor.matmul(ep[c], w2[1][:, c * 128 : (c + 1) * 128], hs[1],
                                     start=False, stop=not has_b2)
                    if has_b2:
                        nc.tensor.matmul(ep[c], b2[:, c * 128 : (c + 1) * 128], magb,
                                         start=False, stop=True)
                ef = []
                for c in range(2):
                    t = work.tile([128, NB], F32, name=f"ef{c}", tag=f"ef{c}")
                    nc.scalar.copy(t, ep[c])
                    ef.append(t)
                    nc.sync.dma_start(out=outE[c * 128 : (c + 1) * 128, cols], in_=t)

                # pre = intervened + eff (fp32); intervened = x with rows 0:K <- iv
                pf = []
                for c in range(2):
                    t = work.tile([128, NB], F32, name=f"pf{c}", tag=f"pf{c}")
                    nc.vector.tensor_add(t, ef[c], xf[c])
                    pf.append(t)
                for (ch, row, off, n) in iv_runs:
                    nc.vector.tensor_add(pf[ch][row:row + n, :],
                                         ef[ch][row:row + n, :],
                                         ivf[off:off + n, :])
                pb = []
                for c in range(2):
                    t = work.tile([128, NB], BF16, name=f"pb{c}", tag=f"pb{c}")
                    nc.scalar.copy(t, pf[c])
                    pb.append(t)

                # cf = pre + pre @ Mc
                cp = [psum.tile([128, NB], F32, name=f"cp{c}", tag="ps", bufs=8)
                      for c in range(2)]
                for c in range(2):
                    nc.tensor.matmul(cp[c], mc[0][:, c * 128 : (c + 1) * 128], pb[0],
                                     start=True, stop=False)
                    nc.tensor.matmul(cp[c], mc[1][:, c * 128 : (c + 1) * 128], pb[1],
                                     start=False, stop=True)
                cf = []
                cb = []
                for c in range(2):
                    t = work.tile([128, NB], F32, name=f"cf{c}", tag=f"cf{c}")
                    nc.vector.tensor_add(t, pf[c], cp[c])
                    cf.append(t)
                    nc.sync.dma_start(out=outC[c * 128 : (c + 1) * 128, cols], in_=t)
                    tb = work.tile([128, NB], BF16, name=f"cb{c}", tag=f"cb{c}")
                    nc.scalar.copy(tb, t)
                    cb.append(tb)

                # intervened (bf16): copy of x chunks with iv rows overwritten.
                # Only chunks containing an intervened variable need a copy.
                touched = sorted({ch for (ch, _, _, _) in iv_runs})
                ib = list(xb)
                for ch in touched:
                    t = work.tile([128, NB], BF16, name=f"ib{ch}", tag=f"ib{ch}")
                    nc.vector.tensor_copy(t, xb[ch])
                    ib[ch] = t
                for (ch, row, off, n) in iv_runs:
                    nc.vector.tensor_copy(ib[ch][row:row + n, :],
                                          ivb[off:off + n, :])

                # plausibility MLP: pin = [x, intervened, cf]
                p1p = psum.tile([128, NB], F32, name="p1p", tag="ps", bufs=8)
                rhs_list = [xb[0], xb[1], ib[0], ib[1], cb[0], cb[1]]
                for i in range(6):
                    nc.tensor.matmul(p1p, pw1[i], rhs_list[i],
                                     start=(i == 0), stop=(i == 5))
                p1b = work.tile([128, NB], BF16, name="p1b", tag="p1b")
                nc.scalar.activation(p1b, p1p, AF.Relu,
                                     bias=pb1 if has_pb1 else 0.0)
                p2p = psum.tile([64, NB], F32, name="p2p", tag="ps", bufs=8)
                nc.tensor.matmul(p2p, pw2, p1b, start=True, stop=True)
                p2b = work.tile([64, NB], BF16, name="p2b", tag="p2b")
                nc.scalar.activation(p2b, p2p, AF.Relu,
                                     bias=pb2 if has_pb2 else 0.0)
                plp = psum.tile([1, NB], F32, name="plp", tag="ps", bufs=8)
                nc.tensor.matmul(plp, pw3, p2b, start=True, stop=True)
                plf = work.tile([1, NB], F32, name="plf", tag="plf")
                nc.scalar.copy(plf, plp)
                nc.sync.dma_start(out=outP[:, cols], in_=plf)

                # impact^2 = sum_v (cf - x)^2  via ones-matmul partition reduction
                df, sq = [], []
                for c in range(2):
                    t = work.tile([128, NB], F32, name=f"df{c}", tag=f"df{c}")
                    nc.vector.tensor_sub(t, cf[c], xf[c])
                    df.append(t)
                    tb = work.tile([128, NB], BF16, name=f"sq{c}", tag=f"sq{c}")
                    nc.scalar.activation(tb, t, AF.Square)
                    sq.append(tb)
                imp = psum.tile([1, NB], F32, name="imp", tag="ps", bufs=8)
                nc.tensor.matmul(imp, ones, sq[0], start=True, stop=False)
                nc.tensor.matmul(imp, ones, sq[1], start=False, stop=True)
                impf = work.tile([1, NB], F32, name="impf", tag="impf")
                nc.scalar.copy(impf, imp)
                nc.sync.dma_start(out=outI[:, cols], in_=impf)

    nc.compile()
    return nc


def _prep_propagation_matrix(cause_idx, effect_idx, strengths, n_rounds=3):
    M = np.eye(V, dtype=np.float64)
    coef = strengths.astype(np.float64) * PROP_SCALE
    c = cause_idx.astype(np.int64)
    e = effect_idx.astype(np.int64)
    for _ in range(n_rounds):
        for i in range(len(c)):
            M[:, e[i]] += coef[i] * M[:, c[i]]
    return M


def kernel(original_data, intervention_values, strengths,
           est_W1, est_b1, est_W2, est_b2,
           pl_W1, pl_b1, pl_W2, pl_b2, pl_W3, pl_b3,
           intervention_variables, cause_idx, effect_idx):
    x = np.ascontiguousarray(np.asarray(original_data, dtype=np.float32))
    iv = np.asarray(intervention_values, dtype=np.float32)
    ivars = np.asarray(intervention_variables, dtype=np.int64)
    B = x.shape[0]
    assert x.shape[1] == V and iv.shape[1] == K
    assert B % (NCORES * NB) == 0, "batch must divide 8*512"
    Bc = B // NCORES

    est_b1 = np.asarray(est_b1, np.float32)
    est_b2 = np.asarray(est_b2, np.float32)
    pl_b1 = np.asarray(pl_b1, np.float32)
    pl_b2 = np.asarray(pl_b2, np.float32)
    pl_b3 = np.asarray(pl_b3, np.float32)
    has_b1 = bool(np.any(est_b1 != 0))
    has_b2 = bool(np.any(est_b2 != 0))
    has_pb1 = bool(np.any(pl_b1 != 0))
    has_pb2 = bool(np.any(pl_b2 != 0))

    # ---- host precompute ----
    M = _prep_propagation_matrix(np.asarray(cause_idx), np.asarray(effect_idx),
                                 np.asarray(strengths, np.float32))
    Mc = (M - np.eye(V)).astype(np.float32)

    W1cat = np.asarray(est_W1, np.float32).reshape(K * 64, V)  # [o, vin]
    W1T = np.ascontiguousarray(W1cat.T).astype(bf16)  # [vin, o]
    W2T = np.ascontiguousarray(
        np.asarray(est_W2, np.float32).transpose(0, 2, 1).reshape(K * 64, V)
    ).astype(bf16)  # [o, vout]
    MC = Mc.astype(bf16)  # [vin, vout]
    SEL = np.zeros((K, V), np.float32)
    for k in range(K):
        SEL[k, k * 64:(k + 1) * 64] = 1.0
    SEL = SEL.astype(bf16)
    PW1T = np.ascontiguousarray(np.asarray(pl_W1, np.float32).T).astype(bf16)
    PW2T = np.ascontiguousarray(np.asarray(pl_W2, np.float32).T).astype(bf16)
    PW3T = np.ascontiguousarray(np.asarray(pl_W3, np.float32).T).astype(bf16)

    xT = np.ascontiguousarray(x.T)  # [V, B]
    xTb = xT.astype(bf16)
    x_iv = x[:, ivars]  # [B, K]
    ivT = np.ascontiguousarray(iv.T)  # [K, B]
    ivTb = ivT.astype(bf16)
    magT = np.ascontiguousarray(np.abs(iv - x_iv).T)  # [K, B]
    magTb = magT.astype(bf16)

    iv_runs = tuple(_iv_runs(ivars.tolist()))
    key = (Bc, has_b1, has_b2, has_pb1, has_pb2, iv_runs)
    if key not in _KERNEL_CACHE:
        _KERNEL_CACHE[key] = _build_nc(*key)
    nc = _KERNEL_CACHE[key]

    shared = {
        "W1T": W1T, "W2T": W2T, "MC": MC, "SEL": SEL,
        "PW1T": PW1T, "PW2T": PW2T, "PW3T": PW3T,
    }
    if has_b1:
        shared["B1"] = np.ascontiguousarray(est_b1.reshape(V, 1))
    if has_b2:
        shared["B2"] = np.ascontiguousarray(est_b2.astype(bf16))  # [K, V]
    if has_pb1:
        shared["PB1"] = np.ascontiguousarray(pl_b1.reshape(128, 1))
    if has_pb2:
        shared["PB2"] = np.ascontiguousarray(pl_b2.reshape(64, 1))

    in_maps = []
    for r in range(NCORES):
        cols = slice(r * Bc, (r + 1) * Bc)
        m = dict(shared)
        m["xT"] = np.ascontiguousarray(xT[:, cols])
        m["xTb"] = np.ascontiguousarray(xTb[:, cols])
        m["ivT"] = np.ascontiguousarray(ivT[:, cols])
        m["ivTb"] = np.ascontiguousarray(ivTb[:, cols])
        m["magTb"] = np.ascontiguousarray(magTb[:, cols])
        in_maps.append(m)

    import os
    trace = bool(int(os.environ.get("CFG_KERNEL_TRACE", "0")))
    res = run_bass_kernel_spmd(nc, in_maps, core_ids=list(range(NCORES)),
                               trace=trace)
    if trace:
        kernel.last_result = res

    final_cf = np.empty((B, V), np.float32)
    effects = np.empty((B, V), np.float32)
    logits = np.empty(B, np.float32)
    imp2 = np.empty(B, np.float32)
    for r in range(NCORES):
        rows = slice(r * Bc, (r + 1) * Bc)
        final_cf[rows] = res.results[r]["outC"].T
        effects[rows] = res.results[r]["outE"].T
        logits[rows] = res.results[r]["outP"][0]
        imp2[rows] = res.results[r]["outI"][0]

    if INTERVENTION_STRENGTH != 1.0:
        effects *= INTERVENTION_STRENGTH
    plaus = 1.0 / (1.0 + np.exp(-(logits + pl_b3.reshape(-1)[0])))
    impact = np.sqrt(np.maximum(imp2, 0.0))
    return (final_cf, effects,
            plaus.astype(np.float32).reshape(B, 1),
            impact.astype(np.float32).reshape(B, 1))
